# revision 1
# baseline (speedup 1.0000x reference)
"""Trainium2 Bass kernel for a dense transformer block (B=4, T=2048, C=1024, 16 heads).

Sharding over 8 NeuronCores: core i handles batch b=i//2 with shard s=i%2.
 - LN1 + QKV + causal attention for its 8 heads (c-slice [512s, 512s+512)) over full T
 - exchange of the peer-half attention outputs via 4 halved ReduceScatter ops
   (zero-masked slots, fully SPMD-symmetric)
 - proj (split local/remote) + LN2 + FFN + residuals on its t-half rows

GEMMs in bf16. LayerNorm gain/bias folded into weights on the host.
Transposes run on the DMA xbar; PSUM tags are double-buffered per phase.
"""

from contextlib import ExitStack

import ml_dtypes
import numpy as np

import concourse.bass as bass
import concourse.mybir as mybir
import concourse.tile as tile
from concourse import bacc
from concourse.bass_utils import run_bass_kernel_spmd

f32 = mybir.dt.float32
f32r = mybir.dt.float32r
bf16 = mybir.dt.bfloat16
AF = mybir.ActivationFunctionType
ALU = mybir.AluOpType
AX = mybir.AxisListType

B, T, C = 4, 2048, 1024
NH, D = 16, 64
F = 4 * C
H = C // 2            # per-core head c-slice (8 heads)
TH = T // 2           # per-core t-half for proj/FFN
EPS = 1e-5
RG = [[0, 1], [2, 3], [4, 5], [6, 7]]

_CACHE = {}


class S:
    """Shared build state."""
    pass


def _layernorm_tile(nc, st, xt, dst, sq_pool, sq_tag):
    """Row-standardize xt [128, C] -> dst [128, C]."""
    work = st.work
    s1 = work.tile([128, 1], f32, name="s1", tag="s1")
    s2 = work.tile([128, 1], f32, name="s2", tag="s2")
    sq = sq_pool.tile([128, C], f32, name="sq", tag=sq_tag)
    nc.vector.reduce_sum(s1[:], xt[:], axis=AX.X)
    nc.scalar.activation(sq[:], xt[:], AF.Square, accum_out=s2[:])
    mu = work.tile([128, 1], f32, name="mu", tag="mu")
    var = work.tile([128, 1], f32, name="var", tag="var")
    nc.vector.tensor_scalar_mul(mu[:], s1[:], 1.0 / C)
    nc.vector.tensor_scalar_mul(s2[:], s2[:], 1.0 / C)
    nc.vector.tensor_tensor(var[:], mu[:], mu[:], ALU.mult)
    nc.vector.tensor_tensor(var[:], s2[:], var[:], ALU.subtract)
    nc.vector.tensor_scalar_add(var[:], var[:], EPS)
    sd = work.tile([128, 1], f32, name="sd", tag="sd")
    nc.scalar.activation(sd[:], var[:], AF.Sqrt)
    rsig = work.tile([128, 1], f32, name="rsig", tag="rsig")
    with nc.allow_low_precision(reason="LN rsqrt"):
        nc.vector.reciprocal(rsig[:], sd[:])
    nmu = work.tile([128, 1], f32, name="nmu", tag="nmu")
    nc.vector.tensor_tensor(nmu[:], mu[:], rsig[:], ALU.mult)
    nc.vector.tensor_scalar_mul(nmu[:], nmu[:], -1.0)
    nc.scalar.activation(dst[:], xt[:], AF.Identity, bias=nmu[:], scale=rsig[:])


def _phase_qkv(nc, st):
    """LN1, xbar transpose, QKV GEMMs. Fills st.qT, st.kT, st.vn."""
    work = st.work
    st.qkvp = st.tc.tile_pool(name="qkv", bufs=1)
    qkv = st.qkvp.__enter__()
    st.wqkvp = st.tc.tile_pool(name="wqkv", bufs=1)
    wqkv = st.wqkvp.__enter__()
    st.xhp = st.tc.tile_pool(name="xh", bufs=2)
    xh = st.xhp.__enter__()
    st.htcp = st.tc.tile_pool(name="htc", bufs=2)
    htc_pool = st.htcp.__enter__()
    ps_qkv_p = st.tc.tile_pool(name="psqkv", bufs=2, space="PSUM")
    ps_qkv = ps_qkv_p.__enter__()

    # x chunk 0 first so LN can start immediately
    xts = []
    for tt in range(4):
        xt = xh.tile([128, C], f32, name="xt", tag=f"xt{tt % 2}")
        nc.sync.dma_start(xt[:], st.x_h[tt * 128:(tt + 1) * 128, :])
        xts.append(xt)

    wq_sb = [wqkv.tile([128, H], bf16, name=f"wq{k}", tag=f"wq{k}") for k in range(8)]
    wk_sb = [wqkv.tile([128, H], bf16, name=f"wk{k}", tag=f"wk{k}") for k in range(8)]
    wv_sb = [wqkv.tile([128, H], bf16, name=f"wv{k}", tag=f"wv{k}") for k in range(8)]
    for k in range(8):
        nc.sync.dma_start(wq_sb[k][:], st.wq_h[k * 128:(k + 1) * 128, :])
    nc.sync.dma_start(st.bq_sb[:], st.bq_h[:])
    for k in range(8):
        nc.sync.dma_start(wk_sb[k][:], st.wk_h[k * 128:(k + 1) * 128, :])
    nc.sync.dma_start(st.bk_sb[:], st.bk_h[:])
    for k in range(8):
        nc.sync.dma_start(wv_sb[k][:], st.wv_h[k * 128:(k + 1) * 128, :])
    for t_, h_ in [(st.bv_sb, st.bv_h), (st.onesr, st.onesr_h),
                   (st.ones8, st.ones8_h),
                   (st.tri, st.tri_h), (st.sel_sb, st.sel_h),
                   (st.seln_sb, st.seln_h), (st.b1_sb, st.b1_h)]:
        nc.sync.dma_start(t_[:], h_[:])

    st.qT = [qkv.tile([128, T], bf16, name=f"qT{i}", tag=f"qT{i}") for i in range(4)]
    st.kT = [qkv.tile([128, T], bf16, name=f"kT{i}", tag=f"kT{i}") for i in range(4)]
    st.vn = [qkv.tile([128, 520], bf16, name=f"vn{i}", tag=f"vn{i}")
             for i in range(16)]

    for j in range(4):  # t-chunks of 512
        # htc[:, tt4, k, :] = transpose block: c-part (within k-chunk), t-free
        htc = htc_pool.tile([128, 4, 8, 128], bf16, name="htc", tag="htc")
        for tt4 in range(4):  # t-tiles of 128 within the chunk
            tt = j * 4 + tt4
            if j == 0:
                xt = xts[tt4]
            else:
                xt = xh.tile([128, C], f32, name="xt", tag=f"xt{tt % 2}")
                nc.sync.dma_start(xt[:], st.x_h[tt * 128:(tt + 1) * 128, :])
            ht = xh.tile([128, C], bf16, name="ht", tag="ht")
            _layernorm_tile(nc, st, xt, ht, xh, "sq")
            nc.sync.dma_start_transpose(htc[:, tt4], ht[:])
        # q/k GEMMs for this t-chunk
        for dst, wsb, bsb in ((st.qT, wq_sb, st.bq_sb), (st.kT, wk_sb, st.bk_sb)):
            for co in range(4):
                pg = ps_qkv.tile([128, 512], f32, name="ps_qk", tag="qk")
                for k in range(8):
                    nc.tensor.matmul(pg[:], wsb[k][:, co * 128:(co + 1) * 128],
                                     htc[:, :, k, :],
                                     start=(k == 0), stop=(k == 7))
                nc.scalar.activation(dst[co][:, j * 512:(j + 1) * 512], pg[:],
                                     AF.Identity, bias=bsb[:, co:co + 1])
        # v GEMM (natural layout, strided into vn with ones columns)
        for tt4 in range(4):
            tt = j * 4 + tt4
            pg = ps_qkv.tile([128, 512], f32, name="ps_v", tag="vv")
            for k in range(8):
                nc.tensor.matmul(pg[:], htc[:, tt4, k, :], wv_sb[k][:],
                                 start=(k == 0), stop=False)
            nc.tensor.matmul(pg[:], st.onesr[:, 0:128], st.bv_sb[:],
                             start=False, stop=True)
            nc.scalar.copy(
                st.vn[tt][:, 0:520].rearrange("p (h e) -> p h e", h=8)[:, :, 0:64],
                pg[:].rearrange("p (h d) -> p h d", h=8))
            nc.sync.dma_start(
                st.vn[tt][:, 0:520].rearrange("p (h e) -> p h e", h=8)[:, :, 64:65],
                st.ones8[:].rearrange("p (h o) -> p h o", h=8))
    ps_qkv_p.__exit__(None, None, None)
    st.htcp.__exit__(None, None, None)
    st.xhp.__exit__(None, None, None)
    st.wqkvp.__exit__(None, None, None)


def _phase_attention(nc, st):
    """Causal attention for 8 local heads; ships results via ReduceScatter."""
    ps_att_p = st.tc.tile_pool(name="psatt", bufs=2, space="PSUM")
    ps_att = ps_att_p.__enter__()
    ps_po_p = st.tc.tile_pool(name="pspo", bufs=1, space="PSUM")
    ps_po = ps_po_p.__enter__()

    st.w1pre_p = st.tc.tile_pool(name="w1pre", bufs=1, side="right")
    w1pre = st.w1pre_p.__enter__()
    st.wop = st.tc.tile_pool(name="wop", bufs=1, side="right")
    wop = st.wop.__enter__()
    st.attp = st.tc.tile_pool(name="attp", bufs=1, side="right")
    attp = st.attp.__enter__()
    st.attsbp = st.tc.tile_pool(name="attsb", bufs=1, side="right")
    attsb = st.attsbp.__enter__()
    st.xrpp = st.tc.tile_pool(name="xrp", bufs=1, side="right")
    xrp = st.xrpp.__enter__()
    aw_p = st.tc.tile_pool(name="aw", bufs=2)
    aw = aw_p.__enter__()

    # prefetches for later phases (independent of attention compute)
    st.wo_sb = [wop.tile([128, C], bf16, name=f"wo{k}", tag=f"wo{k}")
                for k in range(8)]
    for k in range(8):
        nc.sync.dma_start(st.wo_sb[k][:], st.wo_h[k * 128:(k + 1) * 128, :])
    st.xr = [xrp.tile([128, C], f32, name=f"xr{t}", tag=f"xr{t}")
             for t in range(8)]
    for tt in range(8):
        nc.sync.dma_start(st.xr[tt][:], st.xres_h[tt * 128:(tt + 1) * 128, :])
    st.w1g0 = [w1pre.tile([128, 8, 128], bf16, name=f"w1g0_{f}", tag=f"w1g0_{f}")
               for f in range(8)]
    for f in range(8):
        nc.sync.dma_start(st.w1g0[f][:], st.w1_h[f])
    nc.sync.dma_start(st.b2_sb[:], st.b2_h[:])

    st.asb = [[attsb.tile([128, 512], bf16, name=f"asb{k}_{h}",
                          tag=f"asb{k}_{h}") for h in range(2)]
              for k in range(4)]

    attA = [attp.tile([128, T], bf16, name=f"attA{i}", tag=f"attA{i}")
            for i in range(4)]
    st.attA = attA

    pending_tail = [None]

    def flush_tail():
        if pending_tail[0] is not None:
            pending_tail[0]()
            pending_tail[0] = None

    def emit_rs(hp, half):
        nc.gpsimd.collective_compute(
            "ReduceScatter", ALU.add, replica_groups=RG,
            ins=[st.rs_in[hp][half]], outs=[st.rs_out[hp][half]])
        nc.gpsimd.dma_start(st.asb[hp][half][:], st.rs_out[hp][half])

    for hp in range(4):
        for j in range(4):
            tq0 = j * 512
            nk = 4 * (j + 1)
            po = [ps_po.tile([128, 512], f32, tag="po0", name="po0"),
                  ps_po.tile([128, 512], f32, tag="po1", name="po1")]

            def emit_qk(kk):
                r = 128 * (kk - 4 * j) if kk >= 4 * j else 0
                pqk = ps_att.tile([128, 1024], f32, tag="qkp", name="qkp")
                for bi, b0 in enumerate((0, 64)):
                    nc.tensor.matmul(
                        pqk[:, bi * 512 + r:bi * 512 + 512],
                        st.kT[hp][b0:b0 + 64, kk * 128:(kk + 1) * 128],
                        st.qT[hp][b0:b0 + 64, tq0 + r:tq0 + 512],
                        start=True, stop=True)
                return pqk

            pqk_next = emit_qk(0)
            for kk in range(nk):
                r = 128 * (kk - 4 * j) if kk >= 4 * j else 0
                pqk = pqk_next
                if kk + 1 < nk:
                    pqk_next = emit_qk(kk + 1)
                ptb = st.ptp.tile([128, 1024], bf16, name="ptb", tag="pt")
                if r == 0:
                    nc.scalar.activation(ptb[:], pqk[:], AF.Exp)
                else:
                    nc.scalar.activation(
                        ptb[:].rearrange("p (b w) -> p b w", b=2)[:, :, r:512],
                        pqk[:].rearrange("p (b w) -> p b w", b=2)[:, :, r:512],
                        AF.Exp)
                if kk == 0:
                    flush_tail()
                    if j == 3:
                        emit_rs(hp, 0)
                if kk >= 4 * j:
                    nc.vector.tensor_tensor(
                        ptb[:].rearrange("p (b w) -> p b w", b=2)[:, :, r:r + 128],
                        ptb[:].rearrange("p (b w) -> p b w", b=2)[:, :, r:r + 128],
                        st.tri[:, None, :].to_broadcast((128, 2, 128)),
                        ALU.mult)
                for bi in range(2):
                    h = 2 * hp + bi
                    nc.tensor.matmul(
                        po[bi][0:65, r:512],
                        st.vn[kk][:, 65 * h:65 * h + 65],
                        ptb[:, bi * 512 + r:bi * 512 + 512],
                        start=(kk == 0), stop=(kk == nk - 1))
            sj = j // 2
            # softmax tail, deferred past the next j's first QK so the PE
            # queue is not blocked waiting on the reciprocal chain.
            def make_tail(hp=hp, j=j, sj=sj, tq0=tq0, po=po):
                def tail():
                    for bi, b0 in enumerate((0, 64)):
                        rs_row = aw.tile([1, 512], bf16, name="rs_row",
                                         tag=f"rsrow{bi}")
                        nc.vector.tensor_copy(out=rs_row[:], in_=po[bi][64:65, :])
                        pb = ps_po.tile([64, 512], f32, tag=f"pb{bi}",
                                        name=f"pb{bi}")
                        nc.tensor.matmul(pb[:], st.onesr[:, 0:64], rs_row[:],
                                         start=True, stop=True)
                        rbi = aw.tile([64, 512], f32, name="rbi", tag=f"rbi{bi}")
                        with nc.allow_low_precision(reason="softmax denom"):
                            nc.vector.reciprocal_approx_fast(rbi[:], pb[:])
                        nc.vector.scalar_tensor_tensor(
                            attA[hp][b0:b0 + 64, tq0:tq0 + 512],
                            po[bi][0:64, :], st.sel_sb[0:64, sj:sj + 1], rbi[:],
                            ALU.mult, ALU.mult)
                        attBc = aw.tile([64, 512], bf16, name="attBc",
                                        tag=f"attBc{bi}")
                        nc.vector.scalar_tensor_tensor(
                            attBc[:], po[bi][0:64, :],
                            st.seln_sb[0:64, sj:sj + 1], rbi[:],
                            ALU.mult, ALU.mult)
                        nc.sync.dma_start(
                            st.rs_in[hp][j % 2, sj, b0:b0 + 64, :],
                            attBc[:])
                return tail
            pending_tail[0] = make_tail()

        flush_tail()
        emit_rs(hp, 1)

    aw_p.__exit__(None, None, None)
    ps_po_p.__exit__(None, None, None)
    ps_att_p.__exit__(None, None, None)


def _phase_proj(nc, st):
    """Projection + residual, split into a local pass (runs during the last
    ReduceScatter) and a remote pass. Fills st.x2."""
    st.qkvp.__exit__(None, None, None)
    ps_pj_p = st.tc.tile_pool(name="pspj", bufs=2, space="PSUM")
    ps_pj = ps_pj_p.__enter__()
    st.x2p = st.tc.tile_pool(name="x2p", bufs=1)
    x2p = st.x2p.__enter__()
    st.h2p = st.tc.tile_pool(name="h2p", bufs=1)
    h2p = st.h2p.__enter__()
    st.h2wp = st.tc.tile_pool(name="h2w", bufs=2)
    st.h2w = st.h2wp.__enter__()

    st.x2 = [x2p.tile([128, C], f32, name=f"x2_{t}", tag=f"x2_{t}")
             for t in range(8)]
    st.h2c = [h2p.tile([128, 4, 8, 128], bf16, name=f"h2c{i}", tag=f"h2c{i}")
              for i in range(2)]
    st.ps_pj = ps_pj
    st.ps_pj_p = ps_pj_p
    # pass A: local heads only (attA), + residual
    for tt in range(8):
        for cc in range(2):
            pg = ps_pj.tile([128, 512], f32, tag="pj", name="pj")
            for k in range(4):
                for half in range(2):
                    nc.tensor.matmul(
                        pg[:],
                        st.attA[k][:, half * TH + tt * 128:
                                   half * TH + (tt + 1) * 128],
                        st.wo_sb[k][:, cc * 512:(cc + 1) * 512],
                        start=(k == 0 and half == 0), stop=(k == 3 and half == 1))
            nc.vector.tensor_tensor(st.x2[tt][:, cc * 512:(cc + 1) * 512],
                                    pg[:], st.xr[tt][:, cc * 512:(cc + 1) * 512],
                                    ALU.add)
    st.xrpp.__exit__(None, None, None)
    if _CACHE.get("debug"):
        nc.sync.dma_start(st.dq_h[:], st.qT[0][:].bitcast(f32))
        nc.sync.dma_start(st.dk_h[:], st.kT[0][:].bitcast(f32))
        nc.sync.dma_start(st.dv_h[:], st.vn[0][:, 0:520].bitcast(f32))
        nc.sync.dma_start(st.da_h[:], st.attA[0][:].bitcast(f32))
        nc.sync.dma_start(st.dsb_h[:, 0:256], st.asb[0][0][:].bitcast(f32))
        nc.sync.dma_start(st.dsb_h[:, 256:512], st.asb[0][1][:].bitcast(f32))


def _pass_b_half(nc, st, half):
    """Remote-head proj contributions + LN2 + transpose for one t-half."""
    for tt in range(4 * half, 4 * half + 4):
        for cc in range(2):
            pg = st.ps_pj.tile([128, 512], f32, tag="pj", name="pj")
            for k in range(4):
                nc.tensor.matmul(
                    pg[:], st.asb[k][half][:, (tt % 4) * 128:(tt % 4 + 1) * 128],
                    st.wo_sb[4 + k][:, cc * 512:(cc + 1) * 512],
                    start=(k == 0), stop=(k == 3))
            nc.vector.tensor_tensor(st.x2[tt][:, cc * 512:(cc + 1) * 512],
                                    pg[:], st.x2[tt][:, cc * 512:(cc + 1) * 512],
                                    ALU.add)
        h2t = st.h2w.tile([128, C], bf16, name="h2t", tag="h2t")
        _layernorm_tile(nc, st, st.x2[tt], h2t, st.h2w, "sqb")
        nc.sync.dma_start_transpose(st.h2c[half][:, tt % 4], h2t[:])
        if _CACHE.get("debug"):
            nc.sync.dma_start(st.dx2_h[tt * 128:(tt + 1) * 128, :], st.x2[tt][:])


def _open_ffn_pools(nc, st):
    st.ps_f1_p = st.tc.tile_pool(name="psf1", bufs=2, space="PSUM")
    st.ps_f1 = st.ps_f1_p.__enter__()
    st.ps_f2_p = st.tc.tile_pool(name="psf2", bufs=2, space="PSUM")
    st.ps_f2 = st.ps_f2_p.__enter__()
    st.yacp = st.tc.tile_pool(name="yac", bufs=1)
    yac = st.yacp.__enter__()
    st.w1pp = st.tc.tile_pool(name="w1p", bufs=4)
    st.w1p = st.w1pp.__enter__()
    st.w2pp = st.tc.tile_pool(name="w2p", bufs=8)
    st.w2p = st.w2pp.__enter__()
    st.utpp = st.tc.tile_pool(name="utp", bufs=12)
    st.utp = st.utpp.__enter__()
    st.y_acc = [yac.tile([128, C], f32, name=f"ya{t}", tag=f"ya{t}")
                for t in range(8)]


def _phase_ffn_half(nc, st, tch):
    """FFN for one t-half: grouped ff-dim accumulation, residual, output DMA."""
    for g in range(4):
        ut_g = []
        for ff in range(8):
            f = g * 8 + ff
            if g == 0:
                w1c = st.w1g0[ff]
            else:
                w1c = st.w1p.tile([128, 8, 128], bf16, name="w1c", tag="w1c")
                nc.sync.dma_start(w1c[:], st.w1_h[f])
            ut = st.utp.tile([128, 512], bf16, name="ut", tag="ut")
            pg = st.ps_f1.tile([128, 512], f32, tag="f1", name="f1")
            for k in range(8):
                nc.tensor.matmul(pg[:], w1c[:, k, :],
                                 st.h2c[tch][:, :, k, :],
                                 start=(k == 0), stop=(k == 7))
            nc.scalar.activation(ut[:], pg[:], AF.Relu,
                                 bias=st.b1_sb[:, f:f + 1])
            ut_g.append(ut)
        w2g = []
        for ff in range(8):
            f = g * 8 + ff
            w2t = st.w2p.tile([128, C], bf16, name="w2t", tag="w2t")
            nc.sync.dma_start(w2t[:], st.w2_h[f * 128:(f + 1) * 128, :])
            w2g.append(w2t)
        for tt in range(4 * tch, 4 * tch + 4):
            for cc in range(2):
                pg = st.ps_f2.tile([128, 512], f32, tag="f2", name="f2")
                for ff in range(8):
                    nc.tensor.matmul(
                        pg[:], ut_g[ff][:, (tt % 4) * 128:(tt % 4 + 1) * 128],
                        w2g[ff][:, cc * 512:(cc + 1) * 512],
                        start=(ff == 0),
                        stop=(False if g == 0 else ff == 7))
                if g == 0:
                    nc.tensor.matmul(pg[:], st.onesr[:, 0:128],
                                     st.b2_sb[:, cc * 512:(cc + 1) * 512],
                                     start=False, stop=True)
                    nc.vector.tensor_tensor(
                        st.y_acc[tt][:, cc * 512:(cc + 1) * 512], pg[:],
                        st.x2[tt][:, cc * 512:(cc + 1) * 512], ALU.add)
                else:
                    nc.vector.tensor_tensor(
                        st.y_acc[tt][:, cc * 512:(cc + 1) * 512], pg[:],
                        st.y_acc[tt][:, cc * 512:(cc + 1) * 512], ALU.add)
    for tt in range(4 * tch, 4 * tch + 4):
        nc.sync.dma_start(st.y_h[tt * 128:(tt + 1) * 128, :], st.y_acc[tt][:])


def _close_ffn_pools(nc, st):
    st.attsbp.__exit__(None, None, None)
    st.attp.__exit__(None, None, None)
    st.wop.__exit__(None, None, None)
    st.w1pre_p.__exit__(None, None, None)
    st.utpp.__exit__(None, None, None)
    st.w2pp.__exit__(None, None, None)
    st.w1pp.__exit__(None, None, None)
    st.yacp.__exit__(None, None, None)
    st.h2wp.__exit__(None, None, None)
    st.ps_f2_p.__exit__(None, None, None)
    st.ps_f1_p.__exit__(None, None, None)
    st.ps_pj_p.__exit__(None, None, None)
    st.h2p.__exit__(None, None, None)
    st.x2p.__exit__(None, None, None)


def build_program():
    if "nc" in _CACHE:
        return _CACHE["nc"]
    nc = bacc.Bacc(None)
    st = S()

    st.x_h = nc.declare_dram_parameter("x", [T, C], f32, isOutput=False)
    st.xres_h = nc.declare_dram_parameter("xres", [TH, C], f32, isOutput=False)
    st.wq_h = nc.declare_dram_parameter("wq", [C, H], bf16, isOutput=False)
    st.wk_h = nc.declare_dram_parameter("wk", [C, H], bf16, isOutput=False)
    st.wv_h = nc.declare_dram_parameter("wv", [C, H], bf16, isOutput=False)
    st.bq_h = nc.declare_dram_parameter("bq", [128, 4], f32, isOutput=False)
    st.bk_h = nc.declare_dram_parameter("bk", [128, 4], f32, isOutput=False)
    st.bv_h = nc.declare_dram_parameter("bv", [1, H], bf16, isOutput=False)
    st.wo_h = nc.declare_dram_parameter("wo", [C, C], bf16, isOutput=False)
    st.w1_h = nc.declare_dram_parameter("w1", [32, 128, 1024], bf16,
                                        isOutput=False)
    st.b1_h = nc.declare_dram_parameter("b1", [128, 32], f32, isOutput=False)
    st.w2_h = nc.declare_dram_parameter("w2", [F, C], bf16, isOutput=False)
    st.b2_h = nc.declare_dram_parameter("b2", [1, C], bf16, isOutput=False)
    st.tri_h = nc.declare_dram_parameter("tri", [128, 128], bf16, isOutput=False)
    st.onesr_h = nc.declare_dram_parameter("onesr", [1, 128], bf16,
                                           isOutput=False)
    st.ones8_h = nc.declare_dram_parameter("ones8", [128, 8], bf16,
                                           isOutput=False)
    st.sel_h = nc.declare_dram_parameter("sel", [128, 2], f32, isOutput=False)
    st.seln_h = nc.declare_dram_parameter("seln", [128, 2], f32, isOutput=False)
    st.y_h = nc.declare_dram_parameter("y", [TH, C], f32, isOutput=True)
    if _CACHE.get("debug"):
        st.dq_h = nc.declare_dram_parameter("dbg_q", [128, T // 2], f32, isOutput=True)
        st.dk_h = nc.declare_dram_parameter("dbg_k", [128, T // 2], f32, isOutput=True)
        st.dv_h = nc.declare_dram_parameter("dbg_v", [128, 260], f32, isOutput=True)
        st.da_h = nc.declare_dram_parameter("dbg_att", [128, T // 2], f32, isOutput=True)
        st.dsb_h = nc.declare_dram_parameter("dbg_asb", [128, TH // 2], f32, isOutput=True)
        st.dx2_h = nc.declare_dram_parameter("dbg_x2", [TH, C], f32, isOutput=True)

    st.rs_in = [nc.dram_tensor(f"rs_in{hp}", [2, 2, 128, 512], bf16)
                for hp in range(4)]
    st.rs_out = [nc.dram_tensor(f"rs_out{hp}", [2, 128, 512], bf16)
                 for hp in range(4)]

    with tile.TileContext(nc) as tc, ExitStack() as stack:
        st.tc, st.stack = tc, stack
        cst = stack.enter_context(tc.tile_pool(name="const", bufs=1))
        st.work = stack.enter_context(tc.tile_pool(name="work", bufs=4))
        st.ptp = stack.enter_context(tc.tile_pool(name="ptp", bufs=8))

        st.tri = cst.tile([128, 128], bf16, name="tri")
        st.onesr = cst.tile([1, 128], bf16, name="onesr")
        st.ones8 = cst.tile([128, 8], bf16, name="ones8")
        st.bq_sb = cst.tile([128, 4], f32, name="bq_sb")
        st.bk_sb = cst.tile([128, 4], f32, name="bk_sb")
        st.bv_sb = cst.tile([1, H], bf16, name="bv_sb")
        st.b1_sb = cst.tile([128, 32], f32, name="b1_sb")
        st.sel_sb = cst.tile([128, 2], f32, name="sel_sb")
        st.seln_sb = cst.tile([128, 2], f32, name="seln_sb")
        st.b2_sb = cst.tile([1, C], bf16, name="b2_sb")

        _phase_qkv(nc, st)
        _phase_attention(nc, st)
        _phase_proj(nc, st)
        _pass_b_half(nc, st, 0)
        _open_ffn_pools(nc, st)
        _phase_ffn_half(nc, st, 0)
        _pass_b_half(nc, st, 1)
        _phase_ffn_half(nc, st, 1)
        _close_ffn_pools(nc, st)

    nc.compile()
    _CACHE["nc"] = nc
    return nc


def make_inputs(x, Wq, Wk, Wv, Wo, bo, W1, b1, W2, b2,
                ln1_g, ln1_b, ln2_g, ln2_b):
    """Build per-core input maps (host-side sharding + LN folding)."""
    x = np.asarray(x, np.float32)
    scale = float(C) ** -0.5

    wq_eff = ln1_g[:, None] * Wq
    wk_eff = ln1_g[:, None] * Wk * scale
    wv_eff = ln1_g[:, None] * Wv
    bq_full = ln1_b @ Wq
    bk_full = (ln1_b @ Wk) * scale
    bv_full = ln1_b @ Wv
    w1_eff = ln2_g[:, None] * W1
    b1_eff = b1 + ln2_b @ W1

    BF = ml_dtypes.bfloat16
    tri = np.triu(np.ones((128, 128), BF))
    onesr = np.ones((1, 128), BF)
    ones8 = np.ones((128, 8), BF)

    # w1 relayout: w1r[f, p, k*128 + c] = w1_eff[k*128 + p, f*128 + c]
    w1r = np.ascontiguousarray(
        w1_eff.astype(BF).reshape(8, 128, 32, 128).transpose(2, 1, 0, 3)
        .reshape(32, 128, 1024))

    in_maps = []
    for core in range(8):
        b, s = core // 2, core % 2
        cs = slice(s * H, (s + 1) * H)
        ts = slice(s * TH, (s + 1) * TH)
        own = np.arange(s * H, (s + 1) * H)
        other = np.arange((1 - s) * H, (2 - s) * H)
        perm = np.concatenate([own, other])
        in_maps.append({
            "x": np.ascontiguousarray(x[b]),
            "xres": np.ascontiguousarray(x[b, ts, :] + bo[None, :]),
            "wq": np.ascontiguousarray(wq_eff[:, cs].astype(BF)),
            "wk": np.ascontiguousarray(wk_eff[:, cs].astype(BF)),
            "wv": np.ascontiguousarray(wv_eff[:, cs].astype(BF)),
            "bq": np.ascontiguousarray(bq_full[cs].reshape(4, 128).T),
            "bk": np.ascontiguousarray(bk_full[cs].reshape(4, 128).T),
            "bv": np.ascontiguousarray(bv_full[cs].reshape(1, H).astype(BF)),
            "wo": np.ascontiguousarray(Wo[perm, :].astype(BF)),
            "w1": w1r,
            "b1": np.ascontiguousarray(b1_eff.reshape(32, 128).T),
            "w2": np.ascontiguousarray(W2.astype(BF)),
            "b2": np.ascontiguousarray(b2.reshape(1, C).astype(BF)),
            "tri": tri, "onesr": onesr, "ones8": ones8,
            "sel": np.tile(np.eye(2, dtype=np.float32)[s][None, :], (128, 1)),
            "seln": np.tile(np.eye(2, dtype=np.float32)[1 - s][None, :], (128, 1)),
        })
    return in_maps


def kernel(**inputs):
    nc = build_program()
    in_maps = make_inputs(**{k: np.asarray(v, np.float32) for k, v in inputs.items()})
    res = run_bass_kernel_spmd(nc, in_maps, list(range(8)))
    out = np.empty((B, T, C), np.float32)
    for core in range(8):
        b, s = core // 2, core % 2
        out[b, s * TH:(s + 1) * TH, :] = res.results[core]["y"]
    return out

